# revision 9
# baseline (speedup 1.0000x reference)
"""Local (7x7 window) attention kernel for Trainium2, 8 NeuronCores.

Problem: x[8,128,64,64]; q/k/v = 1x1-conv projections of x; attention over the
7x7 spatial neighborhood (zero-padded) summed over channels; softmax over the
49 window positions; y = attn-weighted sum of v over the window.

Sharding: data-parallel over batch B=8 -> one batch element per core.

Per-core strategy (C=128 on SBUF partitions):
  - q,k projections as float32r matmuls with 512-wide moving operands
    (1 cycle/row vs 4 for plain fp32).  k stored h-padded [C,(H+6)*W].
  - v produced transposed, vt[pix,c], off the PE with x pixel-chunks
    stationary; stored bf16 with a 129th all-ones column so the AV matmul
    emits the softmax denominator for free.
  - Scores computed CHUNK-major: for each 2-row key chunk j (128 keys,
    stationary) one f32r matmul against the 4 query blocks j-3..j
    (moving, up to 512 wide) -> S^T[key, q] in PSUM.  35 matmuls total
    instead of 128.
  - Softmax: ACT exp reads score PSUM directly with bias=-30 (uniform
    shift, cancels in softmax; the W-edge denominator correction is
    pre-scaled by e^-30 on the host) -> bf16; out-of-window positions
    killed by a bf16 0/1-mask MULTIPLY on DVE (2-byte 2x mode).  The -30
    shift makes exp overflow impossible, so inf*0 NaNs cannot occur.
  - AV: per query block, 4 bf16 matmuls, em chunk slices stationary,
    vt (128 ch + ones) moving -> pav[q, 129] accumulated in PSUM.
    pav[:,128] is the denominator.
  - normalize: per 3-block group, batched d+dx -> reciprocal, then one
    fused scalar_tensor_tensor per block: y = pav*rd + bv.

Output yT [4096, 128] per core; the host restores [C,H,W].
"""

import sys

if "/opt/trn_rl_repo" not in sys.path:
    sys.path.insert(0, "/opt/trn_rl_repo")

import numpy as np

import concourse.bass as bass
import concourse.bacc as bacc
import concourse.mybir as mybir
from concourse import tile
from concourse.bass_utils import run_bass_kernel_spmd

F32 = mybir.dt.float32
F32R = mybir.dt.float32r
BF16 = mybir.dt.bfloat16

B, C, H, W = 8, 128, 64, 64
KW = 7
PAD = KW // 2            # 3
HP = H + 2 * PAD         # 70 padded rows
NPIX = H * W             # 4096
NPPIX = HP * W           # 4480
RPB = 2                  # query rows per block
NBLK = H // RPB          # 32 blocks
NCHUNK = 4               # key chunks (of 128) per block
NVC = NPPIX // 128       # 35 key/vt chunks
VTW = 129                # vt chunk width: 128 channels + ones column
GRP = 3                  # blocks per normalize batch (3*129 <= 512 psum bank)
SHIFT = 30.0             # uniform score shift before exp (cancels in softmax)

_CACHE = {}


def _build_mask():
    """mask[p, 128*i + qi]: 1 if key (chunk i, within-chunk p) is inside the
    7x7 window of query qi (block-relative), else 0."""
    m = np.zeros((128, NCHUNK * 128), dtype=np.float32)
    for i in range(NCHUNK):
        for p in range(128):
            r, wk = p // 64, p % 64
            for qi in range(128):
                rq, wq = qi // 64, qi % 64
                dh = 2 * i + r - 3 - rq
                if abs(dh) <= PAD and abs(wk - wq) <= PAD:
                    m[p, 128 * i + qi] = 1.0
    return m


def _build_mask_chunkmajor():
    """Chunk-major mask: column group g holds block b = j-3+g, whose
    relative chunk index is i = 3-g."""
    m = _build_mask()
    return np.concatenate(
        [m[:, 128 * (3 - g) : 128 * (4 - g)] for g in range(NCHUNK)], axis=1
    )


def _build_dx():
    """Denominator correction: #window positions outside the image in W,
    per query, times exp(-SHIFT) to match the shifted weights."""
    dx = np.zeros((128, 1), dtype=np.float32)
    for qi in range(128):
        wq = qi % 64
        dx[qi, 0] = float(KW * (max(0, PAD - wq) + max(0, wq - (W - 1 - PAD))))
    return dx * np.exp(np.float32(-SHIFT))


def _build_bass(reps=1):
    nc = bacc.Bacc()

    x_d = nc.dram_tensor("x", [C, NPIX], F32R, kind="ExternalInput")
    wqt_d = nc.dram_tensor("wqt", [C, C], F32R, kind="ExternalInput")
    wkt_d = nc.dram_tensor("wkt", [C, C], F32R, kind="ExternalInput")
    wvt_d = nc.dram_tensor("wvt", [C, C], F32R, kind="ExternalInput")
    bq_d = nc.dram_tensor("bq", [C, 1], F32, kind="ExternalInput")
    bk_d = nc.dram_tensor("bk", [C, 1], F32, kind="ExternalInput")
    bvb_d = nc.dram_tensor("bvb", [128, C], F32, kind="ExternalInput")
    mask_d = nc.dram_tensor("maskT", [128, NCHUNK * 128], BF16, kind="ExternalInput")
    dx_d = nc.dram_tensor("dxcol", [128, 1], F32, kind="ExternalInput")
    y_d = nc.dram_tensor("y", [NPIX, C], F32, kind="ExternalOutput")

    with tile.TileContext(nc) as tc:
        with (
            tc.tile_pool(name="const", bufs=1) as cpool,
            tc.tile_pool(name="big", bufs=1) as bigpool,
            tc.tile_pool(name="sb_er", bufs=3) as sb_er,
            tc.tile_pool(name="sb_y", bufs=4) as sb_y,
        ):
            wqt = cpool.tile([C, C], F32R)
            wkt = cpool.tile([C, C], F32R)
            wvt = cpool.tile([C, C], F32R)
            bq = cpool.tile([C, 1], F32)
            bk = cpool.tile([C, 1], F32)
            maskB = cpool.tile([128, NCHUNK * 128], BF16)
            dxcol = cpool.tile([128, 1], F32)
            bvb = cpool.tile([128, C], F32)
            rd = cpool.tile([128, NBLK], F32)
            negshift = cpool.tile([128, 1], F32)
            nc.gpsimd.memset(negshift[:], -SHIFT)

            x_s = bigpool.tile([C, NPIX + 128], F32R)  # 64 zero cols each side
            q_s = bigpool.tile([C, NPIX], F32R)
            kp_s = bigpool.tile([C, NPPIX], F32R)
            vt_s = bigpool.tile([128, NVC * VTW], BF16)
            em_s = bigpool.tile([128, NVC * 512], BF16)

            nc.sync.dma_start(wqt[:], wqt_d[:])
            nc.sync.dma_start(wkt[:], wkt_d[:])
            nc.sync.dma_start(wvt[:], wvt_d[:])
            nc.sync.dma_start(bq[:], bq_d[:])
            nc.sync.dma_start(bk[:], bk_d[:])
            nc.sync.dma_start(bvb[:], bvb_d[:])
            nc.sync.dma_start(maskB[:], mask_d[:])
            nc.sync.dma_start(dxcol[:], dx_d[:])

            for rep in range(reps):
                for m in range(8):
                    sl = slice(512 * m, 512 * (m + 1))
                    nc.sync.dma_start(x_s[:, 64 + 512 * m : 64 + 512 * (m + 1)], x_d[:, sl])
                nc.gpsimd.memset(x_s[:, 0:64].bitcast(F32), 0.0)
                nc.gpsimd.memset(x_s[:, 64 + NPIX :].bitcast(F32), 0.0)

                # zero-padding of kp and vt; ones columns of vt
                nc.gpsimd.memset(kp_s[:, 0 : PAD * W].bitcast(F32), 0.0)
                nc.gpsimd.memset(kp_s[:, (PAD + H) * W : NPPIX].bitcast(F32), 0.0)
                nc.gpsimd.memset(vt_s[:, 0 : 2 * VTW], 0.0)
                nc.gpsimd.memset(vt_s[:, (NVC - 2) * VTW : NVC * VTW], 0.0)
                ones_cols = vt_s[:, :].rearrange("p (j c) -> p j c", c=VTW)[:, :, 128:129]
                nc.gpsimd.memset(ones_cols, 1.0)

                # ---- q,k projections: f32r, 512-wide moving ----
                with tc.tile_pool(name="ps_qk", bufs=3, space="PSUM") as ps_qk:
                    for m in range(8):
                        sl = slice(512 * m, 512 * (m + 1))
                        pq = ps_qk.tile([128, 512], F32, tag="pqk", name=f"pq{m}")
                        nc.tensor.matmul(
                            pq[:],
                            wqt[:],
                            x_s[:, 64 + 512 * m : 64 + 512 * (m + 1)],
                            start=True,
                            stop=True,
                        )
                        nc.vector.tensor_scalar_add(q_s[:, sl], pq[:], bq[:])
                    for m in range(8):
                        sl = slice(512 * m, 512 * (m + 1))
                        ksl = slice(PAD * W + 512 * m, PAD * W + 512 * (m + 1))
                        pk = ps_qk.tile([128, 512], F32, tag="pqk", name=f"pk{m}")
                        nc.tensor.matmul(
                            pk[:],
                            wkt[:],
                            x_s[:, 64 + 512 * m : 64 + 512 * (m + 1)],
                            start=True,
                            stop=True,
                        )
                        nc.scalar.activation(
                            kp_s[:, ksl],
                            pk[:],
                            mybir.ActivationFunctionType.Identity,
                            bias=bk[:],
                        )

                # ---- v projection, transposed + bf16 + ones column ----
                # vt chunk j covers padded rows (2j, 2j+1); image row h lives at
                # padded row h+3, so chunk j holds image rows (2j-3, 2j-2).
                with tc.tile_pool(name="ps_v", bufs=4, space="PSUM") as ps_v:
                    # f32r matmuls need dst partition 0: the two half-pad
                    # chunks use x_s's zero guard columns so the pad half of
                    # the chunk comes out of the PE as zeros.
                    for j in range(1, NVC - 1):
                        pv = ps_v.tile([128, C], F32, tag="pv", name=f"pv{j}")
                        r0 = 2 * j - 3  # first image row of the chunk (may be -1)
                        lhsT = x_s[:, 64 + r0 * W : 64 + (r0 + 2) * W]
                        dst = vt_s[:, VTW * j : VTW * j + C]
                        nc.tensor.matmul(pv[:], lhsT, wvt[:], start=True, stop=True)
                        if j % 2 == 0:
                            nc.vector.tensor_copy(dst, pv[:])
                        else:
                            nc.scalar.copy(dst, pv[:])

                # ---- attention: chunk-major over key chunks j ----
                ps_s = tc.alloc_tile_pool(name="ps_s", bufs=3, space="PSUM")
                ps_av = tc.alloc_tile_pool(name="ps_av", bufs=3, space="PSUM")
                pav_tiles = {}

                def block_group(b):
                    return b // GRP, b % GRP

                def grp_width(g):
                    return VTW * (min(NBLK, GRP * (g + 1)) - GRP * g)

                def s_phase(j):
                    b0, b1 = max(0, j - 3), min(NBLK - 1, j)
                    nb = b1 - b0 + 1
                    g0 = b0 - (j - 3)           # first valid column group
                    lo, hi = 128 * g0, 128 * (g0 + nb)
                    sps = ps_s.tile([128, 512], F32, tag="sps", name=f"sps{j}")
                    nc.tensor.matmul(
                        sps[:, lo:hi],
                        kp_s[:, 128 * j : 128 * (j + 1)],
                        q_s[:, 128 * b0 : 128 * (b1 + 1)],
                        start=True,
                        stop=True,
                    )
                    er = sb_er.tile([128, 512], BF16, tag="er", name=f"er{j}")
                    nc.scalar.activation(
                        er[:, lo:hi],
                        sps[:, lo:hi],
                        mybir.ActivationFunctionType.Exp,
                        bias=negshift[:],
                    )
                    nc.vector.tensor_mul(
                        em_s[:, 512 * j + lo : 512 * j + hi],
                        er[:, lo:hi],
                        maskB[:, lo:hi],
                    )

                def av_phase(b):
                    g, bb = block_group(b)
                    if bb == 0:
                        pav_tiles[g] = ps_av.tile(
                            [128, grp_width(g)], F32, tag="pav", name=f"pav{g}"
                        )
                    pav = pav_tiles[g][:, VTW * bb : VTW * (bb + 1)]
                    for i in range(NCHUNK):
                        t = b + i
                        em = em_s[:, 512 * t + 128 * (3 - i) : 512 * t + 128 * (4 - i)]
                        nc.tensor.matmul(
                            pav,
                            em,
                            vt_s[:, VTW * t : VTW * (t + 1)],
                            start=(i == 0),
                            stop=(i == NCHUNK - 1),
                        )

                def norm_phase(g):
                    b0 = GRP * g
                    nb = min(NBLK, GRP * (g + 1)) - b0
                    pav = pav_tiles.pop(g)
                    pav3 = pav[:, :].rearrange("p (b c) -> p b c", c=VTW)
                    dcols = pav3[:, :, 128:129]
                    dsum = sb_y.tile([128, GRP], F32, tag="dsum", name=f"dsum{g}")
                    nc.vector.tensor_scalar_add(dsum[:, :nb], dcols, dxcol[:])
                    nc.vector.reciprocal(rd[:, b0 : b0 + nb], dsum[:, :nb])
                    for bb in range(nb):
                        b = b0 + bb
                        ysb = sb_y.tile([128, C], F32, tag="ysb")
                        nc.vector.scalar_tensor_tensor(
                            ysb[:],
                            pav[:, VTW * bb : VTW * bb + C],
                            rd[:, b : b + 1],
                            bvb[:],
                            op0=mybir.AluOpType.mult,
                            op1=mybir.AluOpType.add,
                        )
                        nc.sync.dma_start(y_d[128 * b : 128 * (b + 1), :], ysb[:])

                for j in range(NVC):
                    s_phase(j)
                    if j >= 3:
                        b = j - 3
                        av_phase(b)
                        if b % GRP == GRP - 1 or b == NBLK - 1:
                            norm_phase(b // GRP)
                ps_av.release()
                ps_s.release()

    nc.finalize()
    return nc


def get_nc(reps=1):
    key = ("nc", reps)
    if key not in _CACHE:
        _CACHE[key] = _build_bass(reps)
    return _CACHE[key]


def prepare_in_maps(x, Wq, bq, Wk, bk, Wv, bv):
    x = np.ascontiguousarray(np.asarray(x, dtype=np.float32))
    if "maskb" not in _CACHE:
        bf16 = mybir.dt.np(BF16)
        _CACHE["maskb"] = np.ascontiguousarray(
            _build_mask_chunkmajor().astype(bf16)
        )
        _CACHE["dx"] = _build_dx()
    common = {
        "wqt": np.ascontiguousarray(np.asarray(Wq, np.float32).T),
        "wkt": np.ascontiguousarray(np.asarray(Wk, np.float32).T),
        "wvt": np.ascontiguousarray(np.asarray(Wv, np.float32).T),
        "bq": np.asarray(bq, np.float32).reshape(C, 1),
        "bk": np.asarray(bk, np.float32).reshape(C, 1),
        "bvb": np.ascontiguousarray(
            np.tile(np.asarray(bv, np.float32).reshape(1, C), (128, 1))
        ),
        "maskT": _CACHE["maskb"],
        "dxcol": _CACHE["dx"],
    }
    return [dict(common, x=x[b].reshape(C, NPIX)) for b in range(B)]


def gather_output(results):
    yt = np.stack([results[b]["y"] for b in range(B)])  # [B, 4096, 128]
    return np.ascontiguousarray(yt.transpose(0, 2, 1).reshape(B, C, H, W))


def kernel(x, Wq, bq, Wk, bk, Wv, bv):
    in_maps = prepare_in_maps(x, Wq, bq, Wk, bk, Wv, bv)
    res = run_bass_kernel_spmd(get_nc(), in_maps, list(range(B))).results
    return gather_output(res)


if __name__ == "__main__":
    rng = np.random.default_rng(0)
    xs = rng.standard_normal((B, C, H, W), dtype=np.float32)
    ws = [rng.standard_normal((C, C), dtype=np.float32) / np.sqrt(C) for _ in range(3)]
    bs = [rng.standard_normal(C).astype(np.float32) * 0.01 for _ in range(3)]
    y = kernel(xs, ws[0], bs[0], ws[1], bs[1], ws[2], bs[2])
    print(y.shape, y.dtype)
